# Initial kernel scaffold
#
"""DeepSeek-V3 MLA attention on 8 TRN2 NeuronCores (Bass/Tile).

Self-contained kernel: kernel(**inputs) takes the full unsharded inputs and
returns the full [2, 2048, 2048] float32 output.

Sharding: row-data-parallel projections (each core owns 512 of the 4096
token rows and computes all 16 heads), AllToAll into head-sharded attention
(2 heads per core over all rows), AllToAll back to row-parallel for the
output projection. Activations are kept feature-major so every matmul's
contraction lands on the partition axis; all matmuls run in bf16 with fp32
PSUM accumulation. RoPE pairs are reordered host-side into even/odd column
blocks so the rotation is pure full-tile elementwise work, and the RMSNorm
weights plus softmax scale are folded into the projection weights.
"""
from contextlib import ExitStack

import numpy as np
import ml_dtypes

import concourse.bass as bass
import concourse.mybir as mybir
import concourse.tile as tile
from concourse import bacc
from concourse.bass_utils import run_bass_kernel_spmd

BF16NP = ml_dtypes.bfloat16
SCALE = 192 ** -0.5

dt = mybir.dt
F32, BF16 = dt.float32, dt.bfloat16

P = 128
NC_ = 8
LR = 512               # local rows per core
NH = 16
Q_LORA, KV_LORA = 1536, 512
NLAT = Q_LORA + KV_LORA + 64    # 2112
NOPE, ROPE, VH = 128, 64, 128
EPS = 1e-6
B, S = 2, 2048
R = B * S

# A2A-a shard (V + kpe): V [512 tok, 256 cols] flat + kpe [64, 512]
SHA = 320 * 512
VA_OFF = 0
KPEA_OFF = 256 * 512
# A2A-b shard (Q + K)
SHB = 640 * 512
QN_OFF = 0
QR_OFF = 256 * 512
KN_OFF = 384 * 512
SH2 = 128 * 512


def _blk(dram, j, off, rows, width=512):
    """[rows, width] view at element offset `off` of flat shard j."""
    return dram[j, off:off + rows * width].rearrange("(p c) -> p c", c=width)


def build_kernel(reps: int = 1, debug: bool = False, loopback: bool = False):
    nc = bacc.Bacc(None, target_bir_lowering=False, debug=False)

    XT = nc.dram_tensor("xt", [2048, LR], BF16, kind="ExternalInput")
    WA = nc.dram_tensor("wa", [2048, NLAT], BF16, kind="ExternalInput")
    WQB = nc.dram_tensor("wqb", [Q_LORA, NH * 192], BF16, kind="ExternalInput")
    WKVB = nc.dram_tensor("wkvb", [KV_LORA, NH * 256], BF16, kind="ExternalInput")
    WO = nc.dram_tensor("wo", [2048, 2048], BF16, kind="ExternalInput")
    COST = nc.dram_tensor("cost", [P, LR], F32, kind="ExternalInput")
    SINT = nc.dram_tensor("sint", [P, LR], F32, kind="ExternalInput")
    OUT = nc.dram_tensor("out", [LR, 2048], F32, kind="ExternalOutput")

    SENDA = nc.dram_tensor("senda", [NC_, SHA], BF16, kind="Internal")
    RECVA = nc.dram_tensor("recva", [NC_, SHA], BF16, kind="Internal")
    SENDB = nc.dram_tensor("sendb", [NC_, SHB], BF16, kind="Internal")
    RECVB = nc.dram_tensor("recvb", [NC_, SHB], BF16, kind="Internal")
    SEND2 = [nc.dram_tensor(f"send2{h}", [NC_, SH2], BF16, kind="Internal")
             for h in range(2)]
    RECV2 = [nc.dram_tensor(f"recv2{h}", [NC_, SH2], BF16, kind="Internal")
             for h in range(2)]

    dbg = {}
    if debug:
        dbg["latt"] = nc.dram_tensor("dbg_latt", [P, 17, 512], BF16, kind="ExternalOutput")
        dbg["qt"] = nc.dram_tensor("dbg_qt", [P, 24, 512], BF16, kind="ExternalOutput")
        dbg["kt"] = nc.dram_tensor("dbg_kt", [P, 16, 512], BF16, kind="ExternalOutput")
        dbg["v"] = nc.dram_tensor("dbg_v", [P, 4, 2048], BF16, kind="ExternalOutput")
        dbg["kpe"] = nc.dram_tensor("dbg_kpe", [64, 512], BF16, kind="ExternalOutput")
        dbg["ot"] = nc.dram_tensor("dbg_ot", [P, 2, 4096], BF16, kind="ExternalOutput")

    with tile.TileContext(nc) as tc, ExitStack() as octx:
        consts = octx.enter_context(tc.tile_pool(name="consts", bufs=1))
        ones_bf = consts.tile([P, 1], BF16)
        nc.vector.memset(ones_bf, 1.0)
        ones_f32 = consts.tile([P, 1], F32)
        nc.vector.memset(ones_f32, 1.0)
        masks = consts.tile([P, 4, 512], BF16)
        for m in range(4):
            nc.gpsimd.memset(masks[:, m, :], 1.0)
            # keep where qf - kp - 128m >= 0 else 0
            nc.gpsimd.affine_select(
                out=masks[:, m, :], in_=masks[:, m, :],
                compare_op=mybir.AluOpType.is_ge, fill=0.0,
                base=-128 * m, pattern=[[1, 512]], channel_multiplier=-1,
            )
        eps_t = consts.tile([1, 1], F32)
        nc.vector.memset(eps_t, EPS)
        cos_sb = consts.tile([P, LR], F32)
        sin_sb = consts.tile([P, LR], F32)
        nc.sync.dma_start(out=cos_sb, in_=COST[:, :])
        nc.sync.dma_start(out=sin_sb, in_=SINT[:, :])
        cst = dict(ones_bf=ones_bf, ones_f32=ones_f32, masks=masks,
                   cos=cos_sb, sin=sin_sb, eps=eps_t)

        for rep in range(reps):
            _one_rep(nc, tc, rep, XT, WA, WQB, WKVB, WO, OUT,
                     SENDA, RECVA, SENDB, RECVB, SEND2, RECV2, cst,
                     dbg if rep == reps - 1 else {}, loopback)
    nc.finalize()
    return nc


def _one_rep(nc, tc, rep, XT, WA, WQB, WKVB, WO, OUT,
             SENDA, RECVA, SENDB, RECVB, SEND2, RECV2, cst, dbg,
             loopback=False):
    cos_sb, sin_sb = cst["cos"], cst["sin"]
    ones_bf, ones_f32 = cst["ones_bf"], cst["ones_f32"]
    masks, eps_t = cst["masks"], cst["eps"]

    def _a2a(send, recv):
        if loopback:
            nc.sync.dma_start(out=recv[:, :], in_=send[:, :])
        else:
            nc.gpsimd.collective_compute(
                "AllToAll", mybir.AluOpType.bypass,
                ins=[send[:, :]], outs=[recv[:, :]],
                replica_groups=[list(range(NC_))])

    with ExitStack() as ctx:
      with ExitStack() as pctx:
        s1out = pctx.enter_context(tc.tile_pool(name=f"s1out{rep}", bufs=1))
        latt = s1out.tile([P, 17, 512], BF16)     # lat^T tiles (normed in place)
        kpe_sb = s1out.tile([64, 512], BF16)      # roped k_pe^T (local rows)
        # prefetch wkvb during stage 1 (stage 2 starts with V/K)
        wkvbp = pctx.enter_context(tc.tile_pool(name=f"wkvb{rep}", bufs=1))
        wkvb_sb = wkvbp.tile([P, 4, NH * 256], BF16)
        wkvb_v = WKVB[:, :].rearrange("(kt p) n -> p kt n", p=P)
        for kt in range(4):
            nc.sync.dma_start(out=wkvb_sb[:, kt, :], in_=wkvb_v[:, kt, :])

        # ---------------- Stage 1: lat^T = WA^T @ x^T + rmsnorm stats ------
        with ExitStack() as sctx:
            wap = sctx.enter_context(tc.tile_pool(name=f"wa{rep}", bufs=1))
            pp1 = sctx.enter_context(tc.tile_pool(name=f"ps1{rep}", bufs=4, space="PSUM"))
            ppq = sctx.enter_context(tc.tile_pool(name=f"psq{rep}", bufs=2, space="PSUM"))
            sqp = sctx.enter_context(tc.tile_pool(name=f"sq{rep}", bufs=1))
            nrm = sctx.enter_context(tc.tile_pool(name=f"nrm{rep}", bufs=1))

            wa_sb = wap.tile([P, 16, NLAT], BF16)
            xt_sb = wap.tile([P, 16, LR], BF16)
            wa_v = WA[:, :].rearrange("(kt p) n -> p kt n", p=P)
            xt_v = XT[:, :].rearrange("(kt p) n -> p kt n", p=P)
            for kt in range(16):
                nc.sync.dma_start(out=xt_sb[:, kt, :], in_=xt_v[:, kt, :])
                nc.sync.dma_start(out=wa_sb[:, kt, :], in_=wa_v[:, kt, :])

            sq_all = sqp.tile([P, 16, 512], BF16)
            for pt in range(17):
                pw = 128 if pt < 16 else 64
                ps = pp1.tile([P, 512], F32)
                for kt in range(16):
                    nc.tensor.matmul(
                        ps[:pw, :], lhsT=wa_sb[:, kt, pt * 128:pt * 128 + pw],
                        rhs=xt_sb[:, kt, :], start=(kt == 0), stop=(kt == 15))
                nc.scalar.copy(latt[:pw, pt, :], ps[:pw, :])
                if pt < 16:
                    nc.scalar.square(sq_all[:, pt, :], ps)
            # batched partition-reduction of squares (keeps PE stream dense)
            ps_ssq_q = ppq.tile([1, 512], F32)
            ps_ssq_kv = ppq.tile([1, 512], F32)
            for pt in range(16):
                tgt = ps_ssq_q if pt < 12 else ps_ssq_kv
                nc.tensor.matmul(tgt, lhsT=ones_bf, rhs=sq_all[:, pt, :],
                                 start=(pt in (0, 12)), stop=(pt in (11, 15)))

            # rstd = 1/sqrt(ssq/n + eps), broadcast to 128 partitions
            rq = nrm.tile([1, 512], F32)
            rkv = nrm.tile([1, 512], F32)
            nc.scalar.activation(rq, ps_ssq_q, mybir.ActivationFunctionType.Sqrt,
                                 bias=eps_t, scale=1.0 / Q_LORA)
            nc.scalar.activation(rkv, ps_ssq_kv, mybir.ActivationFunctionType.Sqrt,
                                 bias=eps_t, scale=1.0 / KV_LORA)
            nc.vector.reciprocal(rq, rq)
            nc.vector.reciprocal(rkv, rkv)
            rq_b = nrm.tile([P, 512], F32)
            rkv_b = nrm.tile([P, 512], F32)
            nc.gpsimd.partition_broadcast(rq_b, rq)
            nc.gpsimd.partition_broadcast(rkv_b, rkv)
            for pt in range(12, 16):
                nc.vector.tensor_mul(latt[:, pt, :], latt[:, pt, :], rkv_b)
            for pt in range(12):
                nc.vector.tensor_mul(latt[:, pt, :], latt[:, pt, :], rq_b)

            # k_pe rope (unnormed): latt[:, 16, :] rows [e(32)|o(32)]
            # cross-partition pairs: DMA-shift o-part to partitions 0..31
            kp = nrm.tile([32, 4, 512], F32, tag="krope")
            xo_c = nrm.tile([32, 512], BF16, tag="kxo")
            nc.sync.dma_start(out=xo_c, in_=latt[32:64, 16, :])
            xe = latt[0:32, 16, :]
            c32, s32 = cos_sb[0:32, :], sin_sb[0:32, :]
            nc.vector.tensor_mul(kp[:, 0, :], xe, c32)
            nc.vector.tensor_mul(kp[:, 1, :], xe, s32)
            nc.vector.tensor_mul(kp[:, 2, :], xo_c, s32)
            nc.vector.tensor_mul(kp[:, 3, :], xo_c, c32)
            nc.vector.tensor_sub(kpe_sb[0:32, :], kp[:, 0, :], kp[:, 2, :])
            yi = nrm.tile([32, 512], BF16, tag="kyi")
            nc.vector.tensor_add(yi, kp[:, 1, :], kp[:, 3, :])
            nc.sync.dma_start(out=kpe_sb[32:64, :], in_=yi)

        # kpe send (ready before stage 2)
        for j in range(NC_):
            nc.sync.dma_start(out=_blk(SENDA, j, KPEA_OFF, 64), in_=kpe_sb)

        # ---------------- Stage 2: V -> K -> Q ----------------------------
        s2out = pctx.enter_context(tc.tile_pool(name=f"s2out{rep}", bufs=1))
        with ExitStack() as sctx:
            pp2 = sctx.enter_context(tc.tile_pool(name=f"ps2{rep}", bufs=4, space="PSUM"))

            # V token-major [512, 2048], 4-head groups (N=512 via strided rhs)
            v_sb = s2out.tile([P, 4, 2048], BF16)
            wkvb_g = wkvb_sb.rearrange("p kt (h two vh) -> p kt h two vh",
                                       two=2, vh=128)
            for g in range(4):
                for rt in range(4):
                    ps = pp2.tile([P, 512], F32)
                    rhs = wkvb_g[:, :, 4 * g:4 * g + 4, 1, :]
                    for kt in range(4):
                        nc.tensor.matmul(
                            ps, lhsT=latt[:, 12 + kt, rt * 128:(rt + 1) * 128],
                            rhs=rhs[:, kt, :, :], start=(kt == 0), stop=(kt == 3))
                    nc.scalar.copy(v_sb[:, rt, g * 512:(g + 1) * 512], ps)
            for j in range(NC_):
                for rt in range(4):
                    nc.sync.dma_start(
                        out=_blk(SENDA, j, VA_OFF + rt * 128 * 256, 128, width=256),
                        in_=v_sb[:, rt, 256 * j:256 * j + 256])
            _a2a(SENDA, RECVA)

            # K^T nope [2048, 512]
            kt_sb = s2out.tile([P, 16, 512], BF16)
            for h in range(NH):
                ps = pp2.tile([P, 512], F32)
                for kt in range(4):
                    nc.tensor.matmul(
                        ps, lhsT=wkvb_sb[:, kt, h * 256:h * 256 + 128],
                        rhs=latt[:, 12 + kt, :], start=(kt == 0), stop=(kt == 3))
                nc.scalar.copy(kt_sb[:, h, :], ps)
            for j in range(NC_):
                nc.sync.dma_start(out=_blk(SENDB, j, KN_OFF, 128), in_=kt_sb[:, 2 * j, :])
                nc.sync.dma_start(out=_blk(SENDB, j, KN_OFF + 128 * 512, 128), in_=kt_sb[:, 2 * j + 1, :])

            # Q^T [3072, 512]
            wqbp = sctx.enter_context(tc.tile_pool(name=f"wqb{rep}", bufs=1))
            wqb_sb = wqbp.tile([P, 12, NH * 192], BF16)
            wqb_v = WQB[:, :].rearrange("(kt p) n -> p kt n", p=P)
            for kt in range(12):
                nc.sync.dma_start(out=wqb_sb[:, kt, :], in_=wqb_v[:, kt, :])
            qt_sb = s2out.tile([P, 24, 512], BF16)
            for pt in range(24):
                ps = pp2.tile([P, 512], F32)
                for kt in range(12):
                    nc.tensor.matmul(
                        ps, lhsT=wqb_sb[:, kt, pt * 128:(pt + 1) * 128],
                        rhs=latt[:, kt, :], start=(kt == 0), stop=(kt == 11))
                nc.scalar.copy(qt_sb[:, pt, :], ps)

            # Q rope in place: e-tiles 16+j vs o-tiles 20+j (full-tile ops)
            rp = sctx.enter_context(tc.tile_pool(name=f"qrope{rep}", bufs=2))
            for j in range(4):
                et = qt_sb[:, 16 + j, :]
                ot = qt_sb[:, 20 + j, :]
                t = rp.tile([P, 4, 512], F32, tag="qr")
                nc.vector.tensor_mul(t[:, 0, :], et, cos_sb)
                nc.vector.tensor_mul(t[:, 1, :], et, sin_sb)
                nc.vector.tensor_mul(t[:, 2, :], ot, sin_sb)
                nc.vector.tensor_mul(t[:, 3, :], ot, cos_sb)
                nc.vector.tensor_sub(et, t[:, 0, :], t[:, 2, :])
                nc.vector.tensor_add(ot, t[:, 1, :], t[:, 3, :])

            for j in range(NC_):
                nc.sync.dma_start(out=_blk(SENDB, j, QN_OFF, 128), in_=qt_sb[:, 2 * j, :])
                nc.sync.dma_start(out=_blk(SENDB, j, QN_OFF + 128 * 512, 128), in_=qt_sb[:, 2 * j + 1, :])
                for hi in range(2):
                    h = 2 * j + hi
                    pe = (h % 4) * 32
                    nc.sync.dma_start(
                        out=_blk(SENDB, j, QR_OFF + hi * 64 * 512, 32),
                        in_=qt_sb[pe:pe + 32, 16 + h // 4, :])
                    nc.sync.dma_start(
                        out=_blk(SENDB, j, QR_OFF + (hi * 64 + 32) * 512, 32),
                        in_=qt_sb[pe:pe + 32, 20 + h // 4, :])
            _a2a(SENDB, RECVB)

        if dbg:
            nc.sync.dma_start(out=dbg["latt"][:, :, :], in_=latt)
            nc.sync.dma_start(out=dbg["qt"][:, :, :], in_=qt_sb)
            nc.sync.dma_start(out=dbg["kt"][:, :, :], in_=kt_sb)
            nc.sync.dma_start(out=dbg["v"][:, :, :], in_=v_sb)
            nc.sync.dma_start(out=dbg["kpe"][:, :], in_=kpe_sb)
      # projection pools (latt/wkvb/qt/kt/v) freed here
      if True:
        # ---------------- Stage 4: attention (hl outer, b inner) -----------
        # WO prefetch overlaps attention
        wop = ctx.enter_context(tc.tile_pool(name=f"wo{rep}", bufs=1))
        wo_sb = wop.tile([P, 16, 2048], BF16)
        wo_v = WO[:, :].rearrange("(kt p) n -> p kt n", p=P)
        for kt in range(16):
            nc.sync.dma_start(out=wo_sb[:, kt, :], in_=wo_v[:, kt, :])
        otf = wop.tile([P, 16, 512], BF16)

        with ExitStack() as sctx:
            asm = sctx.enter_context(tc.tile_pool(name=f"asm{rep}", bufs=2))
            ptp = sctx.enter_context(tc.tile_pool(name=f"pt{rep}", bufs=6))
            ppS = sctx.enter_context(tc.tile_pool(name=f"psS{rep}", bufs=3, space="PSUM"))
            ppO = sctx.enter_context(tc.tile_pool(name=f"psO{rep}", bufs=2, space="PSUM"))
            ppD = sctx.enter_context(tc.tile_pool(name=f"psD{rep}", bufs=2, space="PSUM"))
            sml = sctx.enter_context(tc.tile_pool(name=f"sml{rep}", bufs=4))
            otp = sctx.enter_context(tc.tile_pool(name=f"ot{rep}", bufs=1))

            kpool = sctx.enter_context(tc.tile_pool(name=f"kpe{rep}", bufs=1))
            kpe_all = kpool.tile([64, 8, 512], BF16)
            for i in range(NC_):
                nc.sync.dma_start(out=kpe_all[:, i, :], in_=_blk(RECVA, i, KPEA_OFF, 64))

            for hl in range(2):
                ot_sb = otp.tile([P, 4096], BF16, tag=f"ot{hl}")
                for b in range(B):
                    ktn = asm.tile([P, 4, 512], BF16, tag="ktn")
                    qtn = asm.tile([P, 4, 512], BF16, tag="qtn")
                    qtr = asm.tile([64, 4, 512], BF16, tag="qtr")
                    vt = asm.tile([P, 16, 128], BF16, tag="vt")
                    for i in range(4):
                        src = 4 * b + i
                        nc.sync.dma_start(out=ktn[:, i, :], in_=_blk(RECVB, src, KN_OFF + hl * 128 * 512, 128))
                        nc.sync.dma_start(out=qtn[:, i, :], in_=_blk(RECVB, src, QN_OFF + hl * 128 * 512, 128))
                        nc.sync.dma_start(out=qtr[:, i, :], in_=_blk(RECVB, src, QR_OFF + hl * 64 * 512, 64))
                        for rt in range(4):
                            vblk = _blk(RECVA, src, VA_OFF + rt * 128 * 256, 128, width=256)
                            nc.sync.dma_start(
                                out=vt[:, 4 * i + rt, :],
                                in_=vblk[:, hl * 128:(hl + 1) * 128])
                    for qg in range(4):
                        psO = ppO.tile([P, 512], F32)
                        nkt = 4 * qg + 4
                        dacc = sml.tile([P, 512], F32, tag="dacc")
                        for kt in range(nkt):
                            psS = ppS.tile([P, 512], F32)
                            nc.tensor.matmul(
                                psS, lhsT=ktn[:, kt // 4, (kt % 4) * 128:(kt % 4 + 1) * 128],
                                rhs=qtn[:, qg, :], start=True, stop=False)
                            nc.tensor.matmul(
                                psS, lhsT=kpe_all[:, 4 * b + kt // 4, (kt % 4) * 128:(kt % 4 + 1) * 128],
                                rhs=qtr[:, qg, :], start=False, stop=True)
                            pt_t = ptp.tile([P, 512], BF16, tag="pt")
                            nc.scalar.activation(pt_t, psS, mybir.ActivationFunctionType.Exp)
                            if kt >= 4 * qg:
                                nc.vector.tensor_mul(pt_t, pt_t, masks[:, kt - 4 * qg, :])
                            if kt == 0:
                                nc.vector.tensor_copy(dacc, pt_t)
                            else:
                                nc.vector.tensor_add(dacc, dacc, pt_t)
                            nc.tensor.matmul(psO, lhsT=vt[:, kt, :], rhs=pt_t,
                                             start=(kt == 0), stop=(kt == nkt - 1))
                        dacc_bf = sml.tile([P, 512], BF16, tag="daccb")
                        nc.scalar.copy(dacc_bf, dacc)
                        psD = ppD.tile([1, 512], F32)
                        nc.tensor.matmul(psD, lhsT=ones_bf, rhs=dacc_bf,
                                         start=True, stop=True)
                        rcp = sml.tile([1, 512], F32, tag="rcp")
                        nc.vector.reciprocal(rcp, psD)
                        rdb = sml.tile([P, 512], F32, tag="rdb")
                        nc.gpsimd.partition_broadcast(rdb, rcp)
                        nc.vector.tensor_mul(
                            ot_sb[:, b * 2048 + qg * 512:b * 2048 + (qg + 1) * 512],
                            psO, rdb)
                if dbg:
                    nc.sync.dma_start(out=dbg["ot"][:, hl, :], in_=ot_sb)
                # ship this head, overlap with next head's attention
                for j in range(NC_):
                    nc.sync.dma_start(out=_blk(SEND2[hl], j, 0, 128),
                                      in_=ot_sb[:, j * 512:(j + 1) * 512])
                _a2a(SEND2[hl], RECV2[hl])
                for j in range(NC_):
                    nc.sync.dma_start(out=otf[:, 2 * j + hl, :],
                                      in_=_blk(RECV2[hl], j, 0, 128))

        # ---------------- Stage 6: out = O^T.T @ WO ------------------------
        with ExitStack() as sctx:
            pp6 = sctx.enter_context(tc.tile_pool(name=f"ps6{rep}", bufs=4, space="PSUM"))
            outp = sctx.enter_context(tc.tile_pool(name=f"outp{rep}", bufs=3))
            for rt in range(4):
                out_t = outp.tile([P, 2048], F32)
                for ng in range(4):
                    ps = pp6.tile([P, 512], F32)
                    kts = [2 * j for j in range(8)] + [2 * j + 1 for j in range(8)]
                    for i, kt in enumerate(kts):
                        nc.tensor.matmul(
                            ps, lhsT=otf[:, kt, rt * 128:(rt + 1) * 128],
                            rhs=wo_sb[:, kt, ng * 512:(ng + 1) * 512],
                            start=(i == 0), stop=(i == 15))
                    nc.scalar.copy(out_t[:, ng * 512:(ng + 1) * 512], ps)
                nc.sync.dma_start(out=OUT[rt * 128:(rt + 1) * 128, :], in_=out_t)




# ---------------------------------------------------------------------------
# Host-side prep
# ---------------------------------------------------------------------------

def _bf(a):
    return np.asarray(a, dtype=np.float32).astype(BF16NP)


def _prep_weights(wq_a, q_norm_w, wq_b, wkv_a, kv_norm_w, wkv_b, wo,
                  freqs_cos, freqs_sin):
    wkv_a_lat = wkv_a[:, :KV_LORA]
    wkv_a_rope = wkv_a[:, KV_LORA:]
    wkv_a_rope = np.concatenate([wkv_a_rope[:, 0::2], wkv_a_rope[:, 1::2]], axis=1)
    WAh = np.concatenate([wq_a, wkv_a_lat, wkv_a_rope], axis=1)      # [2048, 2112]

    wqb = (wq_b * SCALE) * q_norm_w[:, None]
    wqb = wqb.reshape(Q_LORA, NH, 192)
    nope_cols = wqb[:, :, :NOPE].reshape(Q_LORA, NH * NOPE)
    rope_e = wqb[:, :, NOPE + 0::2].reshape(Q_LORA, NH * 32)
    rope_o = wqb[:, :, NOPE + 1::2].reshape(Q_LORA, NH * 32)
    WQBh = np.concatenate([nope_cols, rope_e, rope_o], axis=1)       # [1536, 3072]

    WKVBh = wkv_b * kv_norm_w[:, None]                                # [512, 4096]
    pos = np.arange(R) % S
    COS = freqs_cos[pos].astype(np.float32)                           # [4096, 32]
    SIN = freqs_sin[pos].astype(np.float32)
    return dict(WA=_bf(WAh), WQB=_bf(WQBh), WKVB=_bf(WKVBh), WO=_bf(wo),
                COS=COS, SIN=SIN)


def _prep_in_maps(inputs):
    x = np.asarray(inputs["x"], dtype=np.float32).reshape(R, 2048)
    W = _prep_weights(
        np.asarray(inputs["wq_a"]), np.asarray(inputs["q_norm_w"]),
        np.asarray(inputs["wq_b"]), np.asarray(inputs["wkv_a"]),
        np.asarray(inputs["kv_norm_w"]), np.asarray(inputs["wkv_b"]),
        np.asarray(inputs["wo"]),
        np.asarray(inputs["freqs_cos"]), np.asarray(inputs["freqs_sin"]))
    in_maps = []
    for c in range(NC_):
        rows = slice(c * LR, (c + 1) * LR)
        in_maps.append({
            "xt": np.ascontiguousarray(x[rows].T).astype(BF16NP),
            "wa": W["WA"], "wqb": W["WQB"], "wkvb": W["WKVB"], "wo": W["WO"],
            "cost": np.ascontiguousarray(np.tile(W["COS"][rows].T, (4, 1))),
            "sint": np.ascontiguousarray(np.tile(W["SIN"][rows].T, (4, 1))),
        })
    return in_maps


_NC_CACHE = []


def _get_nc():
    if not _NC_CACHE:
        _NC_CACHE.append(build_kernel())
    return _NC_CACHE[0]


def kernel(**inputs) -> np.ndarray:
    in_maps = _prep_in_maps(inputs)
    nc = _get_nc()
    res = run_bass_kernel_spmd(nc, in_maps, core_ids=list(range(NC_)))
    outs = [res.results[c]["out"] for c in range(NC_)]
    return np.concatenate(outs, axis=0).reshape(B, S, 2048).astype(np.float32)



# revision 1
# speedup vs baseline: 1.1766x; 1.1766x over previous
"""DeepSeek-V3 MLA attention on 8 TRN2 NeuronCores (Bass/Tile).

Self-contained kernel: kernel(**inputs) takes the full unsharded inputs and
returns the full [2, 2048, 2048] float32 output.

Sharding: row-data-parallel projections (each core owns 512 of the 4096
token rows and computes all 16 heads), AllToAll into head-sharded attention
(2 heads per core over all rows), AllToAll back to row-parallel for the
output projection. Activations are kept feature-major so every matmul's
contraction lands on the partition axis; all matmuls run in bf16 with fp32
PSUM accumulation. RoPE pairs are reordered host-side into even/odd column
blocks so the rotation is pure full-tile elementwise work, and the RMSNorm
weights plus softmax scale are folded into the projection weights.
"""
from contextlib import ExitStack

import numpy as np
import ml_dtypes

import concourse.bass as bass
import concourse.mybir as mybir
import concourse.tile as tile
from concourse import bacc
from concourse.bass_utils import run_bass_kernel_spmd

BF16NP = ml_dtypes.bfloat16
SCALE = 192 ** -0.5

dt = mybir.dt
F32, BF16 = dt.float32, dt.bfloat16

P = 128
NC_ = 8
LR = 512               # local rows per core
NH = 16
Q_LORA, KV_LORA = 1536, 512
NLAT = Q_LORA + KV_LORA + 64    # 2112
NOPE, ROPE, VH = 128, 64, 128
EPS = 1e-6
B, S = 2, 2048
R = B * S

# A2A-a shard (V + kpe): V [512 tok, 256 cols] flat + kpe [64, 512]
SHA = 320 * 512
VA_OFF = 0
KPEA_OFF = 256 * 512
# A2A-b shard (Q + K)
SHB = 640 * 512
QN_OFF = 0
QR_OFF = 256 * 512
KN_OFF = 384 * 512
SH2 = 128 * 512


def _blk(dram, j, off, rows, width=512):
    """[rows, width] view at element offset `off` of flat shard j."""
    return dram[j, off:off + rows * width].rearrange("(p c) -> p c", c=width)


def build_kernel(reps: int = 1, debug: bool = False, loopback: bool = False):
    nc = bacc.Bacc(None, target_bir_lowering=False, debug=False)

    XT = nc.dram_tensor("xt", [2048, LR], BF16, kind="ExternalInput")
    WA = nc.dram_tensor("wa", [2048, NLAT], BF16, kind="ExternalInput")
    WQB = nc.dram_tensor("wqb", [Q_LORA, NH * 192], BF16, kind="ExternalInput")
    WKVB = nc.dram_tensor("wkvb", [KV_LORA, NH * 256], BF16, kind="ExternalInput")
    WO = nc.dram_tensor("wo", [2048, 2048], BF16, kind="ExternalInput")
    COST = nc.dram_tensor("cost", [P, LR], F32, kind="ExternalInput")
    SINT = nc.dram_tensor("sint", [P, LR], F32, kind="ExternalInput")
    OUT = nc.dram_tensor("out", [LR, 2048], F32, kind="ExternalOutput")

    SENDA = nc.dram_tensor("senda", [NC_, SHA], BF16, kind="Internal")
    RECVA = nc.dram_tensor("recva", [NC_, SHA], BF16, kind="Internal")
    SENDB = nc.dram_tensor("sendb", [NC_, SHB], BF16, kind="Internal")
    RECVB = nc.dram_tensor("recvb", [NC_, SHB], BF16, kind="Internal")
    SEND2 = [nc.dram_tensor(f"send2{h}", [NC_, SH2], BF16, kind="Internal")
             for h in range(2)]
    RECV2 = [nc.dram_tensor(f"recv2{h}", [NC_, SH2], BF16, kind="Internal")
             for h in range(2)]

    dbg = {}
    if debug:
        dbg["latt"] = nc.dram_tensor("dbg_latt", [P, 17, 512], BF16, kind="ExternalOutput")
        dbg["qt"] = nc.dram_tensor("dbg_qt", [P, 24, 512], BF16, kind="ExternalOutput")
        dbg["kt"] = nc.dram_tensor("dbg_kt", [P, 16, 512], BF16, kind="ExternalOutput")
        dbg["v"] = nc.dram_tensor("dbg_v", [P, 4, 2048], BF16, kind="ExternalOutput")
        dbg["kpe"] = nc.dram_tensor("dbg_kpe", [64, 512], BF16, kind="ExternalOutput")
        dbg["ot"] = nc.dram_tensor("dbg_ot", [P, 2, 4096], BF16, kind="ExternalOutput")

    with tile.TileContext(nc) as tc, ExitStack() as octx:
        consts = octx.enter_context(tc.tile_pool(name="consts", bufs=1))
        ones_bf = consts.tile([P, 1], BF16)
        nc.vector.memset(ones_bf, 1.0)
        ones_f32 = consts.tile([P, 1], F32)
        nc.vector.memset(ones_f32, 1.0)
        masks = consts.tile([P, 4, 512], BF16)
        for m in range(4):
            nc.gpsimd.memset(masks[:, m, :], 1.0)
            # keep where qf - kp - 128m >= 0 else 0
            nc.gpsimd.affine_select(
                out=masks[:, m, :], in_=masks[:, m, :],
                compare_op=mybir.AluOpType.is_ge, fill=0.0,
                base=-128 * m, pattern=[[1, 512]], channel_multiplier=-1,
            )
        eps_t = consts.tile([1, 1], F32)
        nc.vector.memset(eps_t, EPS)
        cos_sb = consts.tile([P, LR], F32)
        sin_sb = consts.tile([P, LR], F32)
        nc.sync.dma_start(out=cos_sb, in_=COST[:, :])
        nc.sync.dma_start(out=sin_sb, in_=SINT[:, :])
        cst = dict(ones_bf=ones_bf, ones_f32=ones_f32, masks=masks,
                   cos=cos_sb, sin=sin_sb, eps=eps_t)

        for rep in range(reps):
            _one_rep(nc, tc, rep, XT, WA, WQB, WKVB, WO, OUT,
                     SENDA, RECVA, SENDB, RECVB, SEND2, RECV2, cst,
                     dbg if rep == reps - 1 else {}, loopback)
    nc.finalize()
    return nc


def _one_rep(nc, tc, rep, XT, WA, WQB, WKVB, WO, OUT,
             SENDA, RECVA, SENDB, RECVB, SEND2, RECV2, cst, dbg,
             loopback=False):
    cos_sb, sin_sb = cst["cos"], cst["sin"]
    ones_bf, ones_f32 = cst["ones_bf"], cst["ones_f32"]
    masks, eps_t = cst["masks"], cst["eps"]

    def _a2a(send, recv):
        if loopback:
            nc.sync.dma_start(out=recv[:, :], in_=send[:, :])
        else:
            nc.gpsimd.collective_compute(
                "AllToAll", mybir.AluOpType.bypass,
                ins=[send[:, :]], outs=[recv[:, :]],
                replica_groups=[list(range(NC_))])

    with ExitStack() as ctx:
      with ExitStack() as pctx:
        s1out = pctx.enter_context(tc.tile_pool(name=f"s1out{rep}", bufs=1))
        latt = s1out.tile([P, 17, 512], BF16)     # lat^T tiles (normed in place)
        kpe_sb = s1out.tile([64, 512], BF16)      # roped k_pe^T (local rows)
        # prefetch wkvb during stage 1 (stage 2 starts with V/K)
        wkvbp = pctx.enter_context(tc.tile_pool(name=f"wkvb{rep}", bufs=1))
        wkvb_sb = wkvbp.tile([P, 4, NH * 256], BF16)
        wkvb_v = WKVB[:, :].rearrange("(kt p) n -> p kt n", p=P)
        for kt in range(4):
            nc.sync.dma_start(out=wkvb_sb[:, kt, :], in_=wkvb_v[:, kt, :])

        # ---------------- Stage 1: lat^T = WA^T @ x^T + rmsnorm stats ------
        with ExitStack() as sctx:
            wap = sctx.enter_context(tc.tile_pool(name=f"wa{rep}", bufs=1))
            pp1 = sctx.enter_context(tc.tile_pool(name=f"ps1{rep}", bufs=4, space="PSUM"))
            ppq = sctx.enter_context(tc.tile_pool(name=f"psq{rep}", bufs=2, space="PSUM"))
            sqp = sctx.enter_context(tc.tile_pool(name=f"sq{rep}", bufs=1))
            nrm = sctx.enter_context(tc.tile_pool(name=f"nrm{rep}", bufs=1))

            wa_sb = wap.tile([P, 16, NLAT], BF16)
            xt_sb = wap.tile([P, 16, LR], BF16)
            wa_v = WA[:, :].rearrange("(kt p) n -> p kt n", p=P)
            xt_v = XT[:, :].rearrange("(kt p) n -> p kt n", p=P)
            for kt in range(16):
                nc.sync.dma_start(out=xt_sb[:, kt, :], in_=xt_v[:, kt, :])
                nc.sync.dma_start(out=wa_sb[:, kt, :], in_=wa_v[:, kt, :])

            sq_all = sqp.tile([P, 16, 512], BF16)
            for pt in range(17):
                pw = 128 if pt < 16 else 64
                ps = pp1.tile([P, 512], F32)
                for kt in range(16):
                    nc.tensor.matmul(
                        ps[:pw, :], lhsT=wa_sb[:, kt, pt * 128:pt * 128 + pw],
                        rhs=xt_sb[:, kt, :], start=(kt == 0), stop=(kt == 15))
                nc.scalar.copy(latt[:pw, pt, :], ps[:pw, :])
                if pt < 16:
                    nc.scalar.square(sq_all[:, pt, :], ps)
            # batched partition-reduction of squares (keeps PE stream dense)
            ps_ssq_q = ppq.tile([1, 512], F32)
            ps_ssq_kv = ppq.tile([1, 512], F32)
            for pt in range(16):
                tgt = ps_ssq_q if pt < 12 else ps_ssq_kv
                nc.tensor.matmul(tgt, lhsT=ones_bf, rhs=sq_all[:, pt, :],
                                 start=(pt in (0, 12)), stop=(pt in (11, 15)))

            # rstd = 1/sqrt(ssq/n + eps), broadcast to 128 partitions
            rq = nrm.tile([1, 512], F32)
            rkv = nrm.tile([1, 512], F32)
            nc.scalar.activation(rq, ps_ssq_q, mybir.ActivationFunctionType.Sqrt,
                                 bias=eps_t, scale=1.0 / Q_LORA)
            nc.scalar.activation(rkv, ps_ssq_kv, mybir.ActivationFunctionType.Sqrt,
                                 bias=eps_t, scale=1.0 / KV_LORA)
            nc.vector.reciprocal(rq, rq)
            nc.vector.reciprocal(rkv, rkv)
            rq_b = nrm.tile([P, 512], F32)
            rkv_b = nrm.tile([P, 512], F32)
            nc.gpsimd.partition_broadcast(rq_b, rq)
            nc.gpsimd.partition_broadcast(rkv_b, rkv)
            for pt in range(12, 16):
                nc.vector.tensor_mul(latt[:, pt, :], latt[:, pt, :], rkv_b)
            for pt in range(12):
                nc.vector.tensor_mul(latt[:, pt, :], latt[:, pt, :], rq_b)

            # k_pe rope (unnormed): latt[:, 16, :] rows [e(32)|o(32)]
            # cross-partition pairs: DMA-shift o-part to partitions 0..31
            kp = nrm.tile([32, 4, 512], F32, tag="krope")
            xo_c = nrm.tile([32, 512], BF16, tag="kxo")
            nc.sync.dma_start(out=xo_c, in_=latt[32:64, 16, :])
            xe = latt[0:32, 16, :]
            c32, s32 = cos_sb[0:32, :], sin_sb[0:32, :]
            nc.vector.tensor_mul(kp[:, 0, :], xe, c32)
            nc.vector.tensor_mul(kp[:, 1, :], xe, s32)
            nc.vector.tensor_mul(kp[:, 2, :], xo_c, s32)
            nc.vector.tensor_mul(kp[:, 3, :], xo_c, c32)
            nc.vector.tensor_sub(kpe_sb[0:32, :], kp[:, 0, :], kp[:, 2, :])
            yi = nrm.tile([32, 512], BF16, tag="kyi")
            nc.vector.tensor_add(yi, kp[:, 1, :], kp[:, 3, :])
            nc.sync.dma_start(out=kpe_sb[32:64, :], in_=yi)

        # kpe send (ready before stage 2)
        for j in range(NC_):
            nc.sync.dma_start(out=_blk(SENDA, j, KPEA_OFF, 64), in_=kpe_sb)

        # ---------------- Stage 2: V -> K -> Q ----------------------------
        s2out = pctx.enter_context(tc.tile_pool(name=f"s2out{rep}", bufs=1))
        with ExitStack() as sctx:
            pp2 = sctx.enter_context(tc.tile_pool(name=f"ps2{rep}", bufs=4, space="PSUM"))

            # V token-major [512, 2048], 4-head groups (N=512 via strided rhs)
            v_sb = s2out.tile([P, 4, 2048], BF16)
            wkvb_g = wkvb_sb.rearrange("p kt (h two vh) -> p kt h two vh",
                                       two=2, vh=128)
            for g in range(4):
                for rt in range(4):
                    ps = pp2.tile([P, 512], F32)
                    rhs = wkvb_g[:, :, 4 * g:4 * g + 4, 1, :]
                    for kt in range(4):
                        nc.tensor.matmul(
                            ps, lhsT=latt[:, 12 + kt, rt * 128:(rt + 1) * 128],
                            rhs=rhs[:, kt, :, :], start=(kt == 0), stop=(kt == 3))
                    nc.scalar.copy(v_sb[:, rt, g * 512:(g + 1) * 512], ps)
            for j in range(NC_):
                for rt in range(4):
                    nc.sync.dma_start(
                        out=_blk(SENDA, j, VA_OFF + rt * 128 * 256, 128, width=256),
                        in_=v_sb[:, rt, 256 * j:256 * j + 256])
            _a2a(SENDA, RECVA)

            # K^T nope [2048, 512]
            kt_sb = s2out.tile([P, 16, 512], BF16)
            for h in range(NH):
                ps = pp2.tile([P, 512], F32)
                for kt in range(4):
                    nc.tensor.matmul(
                        ps, lhsT=wkvb_sb[:, kt, h * 256:h * 256 + 128],
                        rhs=latt[:, 12 + kt, :], start=(kt == 0), stop=(kt == 3))
                nc.scalar.copy(kt_sb[:, h, :], ps)
            for j in range(NC_):
                nc.sync.dma_start(out=_blk(SENDB, j, KN_OFF, 128), in_=kt_sb[:, 2 * j, :])
                nc.sync.dma_start(out=_blk(SENDB, j, KN_OFF + 128 * 512, 128), in_=kt_sb[:, 2 * j + 1, :])

            # Q^T [3072, 512]
            wqbp = sctx.enter_context(tc.tile_pool(name=f"wqb{rep}", bufs=1))
            wqb_sb = wqbp.tile([P, 12, NH * 192], BF16)
            wqb_v = WQB[:, :].rearrange("(kt p) n -> p kt n", p=P)
            for kt in range(12):
                nc.sync.dma_start(out=wqb_sb[:, kt, :], in_=wqb_v[:, kt, :])
            qt_sb = s2out.tile([P, 24, 512], BF16)
            for pt in range(24):
                ps = pp2.tile([P, 512], F32)
                for kt in range(12):
                    nc.tensor.matmul(
                        ps, lhsT=wqb_sb[:, kt, pt * 128:(pt + 1) * 128],
                        rhs=latt[:, kt, :], start=(kt == 0), stop=(kt == 11))
                nc.scalar.copy(qt_sb[:, pt, :], ps)

            # Q rope in place: e-tiles 16+j vs o-tiles 20+j (full-tile ops)
            rp = sctx.enter_context(tc.tile_pool(name=f"qrope{rep}", bufs=2))
            for j in range(4):
                et = qt_sb[:, 16 + j, :]
                ot = qt_sb[:, 20 + j, :]
                t = rp.tile([P, 4, 512], F32, tag="qr")
                nc.vector.tensor_mul(t[:, 0, :], et, cos_sb)
                nc.vector.tensor_mul(t[:, 1, :], et, sin_sb)
                nc.vector.tensor_mul(t[:, 2, :], ot, sin_sb)
                nc.vector.tensor_mul(t[:, 3, :], ot, cos_sb)
                nc.vector.tensor_sub(et, t[:, 0, :], t[:, 2, :])
                nc.vector.tensor_add(ot, t[:, 1, :], t[:, 3, :])

            for j in range(NC_):
                nc.sync.dma_start(out=_blk(SENDB, j, QN_OFF, 128), in_=qt_sb[:, 2 * j, :])
                nc.sync.dma_start(out=_blk(SENDB, j, QN_OFF + 128 * 512, 128), in_=qt_sb[:, 2 * j + 1, :])
                for hi in range(2):
                    h = 2 * j + hi
                    pe = (h % 4) * 32
                    nc.sync.dma_start(
                        out=_blk(SENDB, j, QR_OFF + hi * 64 * 512, 32),
                        in_=qt_sb[pe:pe + 32, 16 + h // 4, :])
                    nc.sync.dma_start(
                        out=_blk(SENDB, j, QR_OFF + (hi * 64 + 32) * 512, 32),
                        in_=qt_sb[pe:pe + 32, 20 + h // 4, :])
            _a2a(SENDB, RECVB)

        if dbg:
            nc.sync.dma_start(out=dbg["latt"][:, :, :], in_=latt)
            nc.sync.dma_start(out=dbg["qt"][:, :, :], in_=qt_sb)
            nc.sync.dma_start(out=dbg["kt"][:, :, :], in_=kt_sb)
            nc.sync.dma_start(out=dbg["v"][:, :, :], in_=v_sb)
            nc.sync.dma_start(out=dbg["kpe"][:, :], in_=kpe_sb)
      # projection pools (latt/wkvb/qt/kt/v) freed here
      if True:
        # ---------------- Stage 4: attention (hl outer, b inner) -----------
        # WO prefetch overlaps attention
        wop = ctx.enter_context(tc.tile_pool(name=f"wo{rep}", bufs=1))
        wo_sb = wop.tile([P, 16, 2048], BF16)
        wo_v = WO[:, :].rearrange("(kt p) n -> p kt n", p=P)
        for kt in range(16):
            nc.sync.dma_start(out=wo_sb[:, kt, :], in_=wo_v[:, kt, :])
        otf = wop.tile([P, 16, 512], BF16)

        with ExitStack() as sctx:
            asm = sctx.enter_context(tc.tile_pool(name=f"asm{rep}", bufs=2))
            ptp = sctx.enter_context(tc.tile_pool(name=f"pt{rep}", bufs=6))
            ppS = sctx.enter_context(tc.tile_pool(name=f"psS{rep}", bufs=3, space="PSUM"))
            ppO = sctx.enter_context(tc.tile_pool(name=f"psO{rep}", bufs=2, space="PSUM"))
            ppD = sctx.enter_context(tc.tile_pool(name=f"psD{rep}", bufs=2, space="PSUM"))
            sml = sctx.enter_context(tc.tile_pool(name=f"sml{rep}", bufs=4))
            otp = sctx.enter_context(tc.tile_pool(name=f"ot{rep}", bufs=1))

            kpool = sctx.enter_context(tc.tile_pool(name=f"kpe{rep}", bufs=1))
            kpe_all = kpool.tile([64, 8, 512], BF16)
            for i in range(NC_):
                nc.sync.dma_start(out=kpe_all[:, i, :], in_=_blk(RECVA, i, KPEA_OFF, 64))

            for hl in range(2):
                ot_sb = otp.tile([P, 4096], BF16, tag=f"ot{hl}")
                for b in range(B):
                    ktn = asm.tile([P, 4, 512], BF16, tag="ktn")
                    qtn = asm.tile([P, 4, 512], BF16, tag="qtn")
                    qtr = asm.tile([64, 4, 512], BF16, tag="qtr")
                    vt = asm.tile([P, 16, 128], BF16, tag="vt")
                    for i in range(4):
                        src = 4 * b + i
                        nc.sync.dma_start(out=ktn[:, i, :], in_=_blk(RECVB, src, KN_OFF + hl * 128 * 512, 128))
                        nc.sync.dma_start(out=qtn[:, i, :], in_=_blk(RECVB, src, QN_OFF + hl * 128 * 512, 128))
                        nc.sync.dma_start(out=qtr[:, i, :], in_=_blk(RECVB, src, QR_OFF + hl * 64 * 512, 64))
                        for rt in range(4):
                            vblk = _blk(RECVA, src, VA_OFF + rt * 128 * 256, 128, width=256)
                            nc.sync.dma_start(
                                out=vt[:, 4 * i + rt, :],
                                in_=vblk[:, hl * 128:(hl + 1) * 128])
                    for qg in range(4):
                        psO = ppO.tile([P, 512], F32)
                        nkt = 4 * qg + 4
                        dacc = sml.tile([P, 512], F32, tag="dacc")
                        for kt in range(nkt):
                            psS = ppS.tile([P, 512], F32)
                            nc.tensor.matmul(
                                psS, lhsT=ktn[:, kt // 4, (kt % 4) * 128:(kt % 4 + 1) * 128],
                                rhs=qtn[:, qg, :], start=True, stop=False)
                            nc.tensor.matmul(
                                psS, lhsT=kpe_all[:, 4 * b + kt // 4, (kt % 4) * 128:(kt % 4 + 1) * 128],
                                rhs=qtr[:, qg, :], start=False, stop=True)
                            pt_t = ptp.tile([P, 512], BF16, tag="pt")
                            nc.scalar.activation(pt_t, psS, mybir.ActivationFunctionType.Exp)
                            if kt >= 4 * qg:
                                nc.vector.tensor_mul(pt_t, pt_t, masks[:, kt - 4 * qg, :])
                            if kt == 0:
                                nc.vector.tensor_copy(dacc, pt_t)
                            else:
                                nc.vector.tensor_add(dacc, dacc, pt_t)
                            nc.tensor.matmul(psO, lhsT=vt[:, kt, :], rhs=pt_t,
                                             start=(kt == 0), stop=(kt == nkt - 1))
                        dacc_bf = sml.tile([P, 512], BF16, tag="daccb")
                        nc.scalar.copy(dacc_bf, dacc)
                        psD = ppD.tile([1, 512], F32)
                        nc.tensor.matmul(psD, lhsT=ones_bf, rhs=dacc_bf,
                                         start=True, stop=True)
                        rcp = sml.tile([1, 512], F32, tag="rcp")
                        nc.vector.reciprocal(rcp, psD)
                        rdb = sml.tile([P, 512], F32, tag="rdb")
                        nc.gpsimd.partition_broadcast(rdb, rcp)
                        nc.vector.tensor_mul(
                            ot_sb[:, b * 2048 + qg * 512:b * 2048 + (qg + 1) * 512],
                            psO, rdb)
                if dbg:
                    nc.sync.dma_start(out=dbg["ot"][:, hl, :], in_=ot_sb)
                # ship this head, overlap with next head's attention
                for j in range(NC_):
                    nc.sync.dma_start(out=_blk(SEND2[hl], j, 0, 128),
                                      in_=ot_sb[:, j * 512:(j + 1) * 512])
                _a2a(SEND2[hl], RECV2[hl])
                for j in range(NC_):
                    nc.sync.dma_start(out=otf[:, 2 * j + hl, :],
                                      in_=_blk(RECV2[hl], j, 0, 128))

        # ---------------- Stage 6: out = O^T.T @ WO ------------------------
        with ExitStack() as sctx:
            pp6 = sctx.enter_context(tc.tile_pool(name=f"ps6{rep}", bufs=4, space="PSUM"))
            outp = sctx.enter_context(tc.tile_pool(name=f"outp{rep}", bufs=3))
            for rt in range(4):
                out_t = outp.tile([P, 2048], F32)
                for ng in range(4):
                    ps = pp6.tile([P, 512], F32)
                    kts = [2 * j for j in range(8)] + [2 * j + 1 for j in range(8)]
                    for i, kt in enumerate(kts):
                        nc.tensor.matmul(
                            ps, lhsT=otf[:, kt, rt * 128:(rt + 1) * 128],
                            rhs=wo_sb[:, kt, ng * 512:(ng + 1) * 512],
                            start=(i == 0), stop=(i == 15))
                    nc.scalar.copy(out_t[:, ng * 512:(ng + 1) * 512], ps)
                nc.sync.dma_start(out=OUT[rt * 128:(rt + 1) * 128, :], in_=out_t)




# ---------------------------------------------------------------------------
# Host-side prep
# ---------------------------------------------------------------------------

def _bf(a):
    return np.asarray(a, dtype=np.float32).astype(BF16NP)


def _prep_weights(wq_a, q_norm_w, wq_b, wkv_a, kv_norm_w, wkv_b, wo,
                  freqs_cos, freqs_sin):
    wkv_a_lat = wkv_a[:, :KV_LORA]
    wkv_a_rope = wkv_a[:, KV_LORA:]
    wkv_a_rope = np.concatenate([wkv_a_rope[:, 0::2], wkv_a_rope[:, 1::2]], axis=1)
    WAh = np.concatenate([wq_a, wkv_a_lat, wkv_a_rope], axis=1)      # [2048, 2112]

    wqb = (wq_b * SCALE) * q_norm_w[:, None]
    wqb = wqb.reshape(Q_LORA, NH, 192)
    nope_cols = wqb[:, :, :NOPE].reshape(Q_LORA, NH * NOPE)
    rope_e = wqb[:, :, NOPE + 0::2].reshape(Q_LORA, NH * 32)
    rope_o = wqb[:, :, NOPE + 1::2].reshape(Q_LORA, NH * 32)
    WQBh = np.concatenate([nope_cols, rope_e, rope_o], axis=1)       # [1536, 3072]

    WKVBh = wkv_b * kv_norm_w[:, None]                                # [512, 4096]
    pos = np.arange(R) % S
    COS = freqs_cos[pos].astype(np.float32)                           # [4096, 32]
    SIN = freqs_sin[pos].astype(np.float32)
    return dict(WA=_bf(WAh), WQB=_bf(WQBh), WKVB=_bf(WKVBh), WO=_bf(wo),
                COS=COS, SIN=SIN)


def _prep_in_maps(inputs):
    x = np.asarray(inputs["x"], dtype=np.float32).reshape(R, 2048)
    W = _prep_weights(
        np.asarray(inputs["wq_a"]), np.asarray(inputs["q_norm_w"]),
        np.asarray(inputs["wq_b"]), np.asarray(inputs["wkv_a"]),
        np.asarray(inputs["kv_norm_w"]), np.asarray(inputs["wkv_b"]),
        np.asarray(inputs["wo"]),
        np.asarray(inputs["freqs_cos"]), np.asarray(inputs["freqs_sin"]))
    in_maps = []
    for c in range(NC_):
        rows = slice(c * LR, (c + 1) * LR)
        in_maps.append({
            "xt": np.ascontiguousarray(x[rows].T).astype(BF16NP),
            "wa": W["WA"], "wqb": W["WQB"], "wkvb": W["WKVB"], "wo": W["WO"],
            "cost": np.ascontiguousarray(np.tile(W["COS"][rows].T, (4, 1))),
            "sint": np.ascontiguousarray(np.tile(W["SIN"][rows].T, (4, 1))),
        })
    return in_maps


_NC_CACHE = []


def _get_nc():
    if not _NC_CACHE:
        _NC_CACHE.append(build_kernel())
    return _NC_CACHE[0]


def kernel(**inputs) -> np.ndarray:
    in_maps = _prep_in_maps(inputs)
    nc = _get_nc()
    res = run_bass_kernel_spmd(nc, in_maps, core_ids=list(range(NC_)))
    outs = [res.results[c]["out"] for c in range(NC_)]
    return np.concatenate(outs, axis=0).reshape(B, S, 2048).astype(np.float32)

